# revision 2
# baseline (speedup 1.0000x reference)
"""CharLevelEncoder Trainium2 kernel (8-core SPMD), v3.

Math: out = relu(concat(word_emb[word_ids], h(char_ids)) @ W_lin.T + b_lin)
with h a single LSTM cell step from zero state on E[char_ids].

Algebraic restructuring (as v2):
  - h depends only on char_id (40 values) -> HB table [40, WD]:
        HB[c] = h_c @ W_lin[:, WD:].T + b_lin
  - word_emb[word_ids] @ W_lin[:, :WD].T == (word_emb @ A.T)[word_ids]
  - out[t] = relu(P[word_ids[t]] + HB[char_ids[t]])

v3 change vs v2: the expansion uses ONE matmul per PSUM tile instead of
two.  Words are packed into bins of exactly 128 chars and <=32 words;
every bin owns a uniform 32-row window [32b, 32b+32) of the core's
P-row layout (dummy slots padded with zero word vectors).  Each bin's
expansion rhs is a persistent SBUF tile whose rows 0..39 hold HB
(stamped once) and rows 40..71 get the bin's P window via a single
SBUF->SBUF DMA.  The lhsT is a TWO-hot fp8 matrix: column (char) has a
one at row charid and a one at row 40+slot, so gather + HB-add +
accumulate happen in a single 512-col matmul pass.  Expansion PE cost
halves: 2x512 feeds per 128-char tile instead of 4x512.

PE cost model (cols x 0.417ns): P-GEMM 32 chunks x 8192 = 109us,
expansion 128 bins x 1024 = 55us -> ~165us PE per core.
"""

import os

import ml_dtypes
import numpy as np

import concourse.bass as bass
import concourse.tile as tile
from concourse import bacc, mybir
from concourse.bass_utils import run_bass_kernel_spmd

NCORES = 8
WD = 1024
NE = 40
HID = 512
WS = 32           # word slots per bin
CAP = 128         # chars per bin (one 128-col tile per bin)
KC = NE + 2 * WS  # expansion contraction rows (104): HB + TWO bins' windows
NSLOT = 8         # persistent expansion-rhs tiles (2 bins each)
GB = 16           # bins per one-hot DMA block

NPBF = ml_dtypes.bfloat16
NP16 = np.float16
NP8 = ml_dtypes.float8_e4m3


def _sigmoid(x):
    return 1.0 / (1.0 + np.exp(-x))


def _hb_table(E, W_ih, b_ih, b_hh, W_lin, b_lin):
    G = E.astype(np.float32) @ W_ih.T + b_ih + b_hh  # [NE, 4H]
    i, f, g, o = np.split(G, 4, axis=1)
    c = _sigmoid(i) * np.tanh(g)
    h = _sigmoid(o) * np.tanh(c)  # [NE, H]
    return (h @ W_lin[:, WD:].T + b_lin).astype(np.float32)  # [NE, WD]


def _repair(bin_of, c, nb):
    """Rebalance snake-dealt bins until every bin has <=CAP chars, via
    word moves (when a destination has room) or 1-for-1 swaps.  The
    potential sum(max(sums-CAP,0)) strictly decreases per operation."""
    sums = np.bincount(bin_of, weights=c, minlength=nb).astype(np.int64)
    nw = np.bincount(bin_of, minlength=nb).astype(np.int64)
    if nw.max() > WS:
        return False
    smax = int(c.max())
    cnt_bs = np.zeros((nb, smax + 1), np.int64)
    words_in = {b: [] for b in range(nb)}
    for i, b in enumerate(bin_of):
        words_in[int(b)].append(i)
        cnt_bs[int(b), int(c[i])] += 1
    for _ in range(400000):
        bo = int(np.argmax(sums))
        e = int(sums[bo]) - CAP
        if e <= 0:
            return True
        cand = sorted(words_in[bo], key=lambda i: int(c[i]))
        a = None
        for i in reversed(cand):  # largest word with count <= excess
            if c[i] <= e:
                a = i
                break
        moved = False
        if a is not None:
            ca = int(c[a])
            mask = (sums + ca <= CAP) & (nw < WS)
            mask[bo] = False
            if mask.any():
                dest = int(np.argmax(np.where(mask, sums, -1)))  # tightest
                words_in[bo].remove(a)
                words_in[dest].append(a)
                bin_of[a] = dest
                sums[bo] -= ca
                sums[dest] += ca
                nw[bo] -= 1
                nw[dest] += 1
                cnt_bs[bo, ca] -= 1
                cnt_bs[dest, ca] += 1
                moved = True
        if not moved:
            done = False
            for a2 in reversed(cand):
                ca2 = int(c[a2])
                for delta in range(min(e, ca2 - 1), 0, -1):
                    t = ca2 - delta
                    m = (cnt_bs[:, t] > 0) & (sums <= CAP - delta)
                    m[bo] = False
                    if not m.any():
                        continue
                    bb = int(np.argmax(np.where(m, sums, -1)))
                    b2 = next(i for i in words_in[bb] if c[i] == t)
                    words_in[bo].remove(a2)
                    words_in[bb].append(a2)
                    words_in[bb].remove(b2)
                    words_in[bo].append(b2)
                    bin_of[a2] = bb
                    bin_of[b2] = bo
                    sums[bo] -= delta
                    sums[bb] += delta
                    cnt_bs[bo, ca2] -= 1
                    cnt_bs[bb, ca2] += 1
                    cnt_bs[bb, t] -= 1
                    cnt_bs[bo, t] += 1
                    done = True
                    break
                if done:
                    break
            if not done:
                return False
    return False


def _pack_bins(word_ids, n_words):
    """Pack used words into bins: <=CAP chars and <=WS words per bin.
    Returns (bin_of_word, slot_of_word, nbins); nbins % 32 == 0 so each
    core gets nbins/8 bins with (nbins/8)*WS % 128 == 0."""
    wc = np.bincount(word_ids, minlength=n_words)
    used = np.nonzero(wc)[0]
    counts = wc[used].astype(np.int64)
    order = np.argsort(-counts, kind="stable")
    wsorted = used[order]
    csorted = counts[order]
    nused = len(used)
    assert csorted.max() <= CAP, "single word exceeds bin capacity"

    nb = max(1024, int(np.ceil(nused / WS)))
    nb = ((nb + 31) // 32) * 32
    while True:
        bin_of = np.empty(nused, np.int32)
        for r in range(int(np.ceil(nused / nb))):
            lo, hi = r * nb, min((r + 1) * nb, nused)
            idx = np.arange(lo, hi)
            bin_of[idx] = (idx - lo) if r % 2 == 0 else (nb - 1 - (idx - lo))
        if _repair(bin_of, csorted, nb):
            break
        nb += 32

    bin_of_word = np.full(n_words, -1, np.int32)
    bin_of_word[wsorted] = bin_of
    ord2 = np.argsort(bin_of, kind="stable")
    starts = np.concatenate([[0], np.cumsum(np.bincount(bin_of, minlength=nb))])
    slots = np.arange(nused) - starts[bin_of[ord2]]
    slot_of_word = np.full(n_words, -1, np.int32)
    slot_of_word[wsorted[ord2]] = slots
    return bin_of_word, slot_of_word, nb


def _build_program(B, nch):
    """B bins per core (one 128-char tile each), nch = B*WS/128 P chunks.
    Bin pairs share an rhs slot tile: rows 0..39 HB, 40..71 bin 2p,
    72..103 bin 2p+1 -> one 64-row regroup DMA per pair."""
    f32 = mybir.dt.float32
    f16 = mybir.dt.float16
    bf16 = mybir.dt.bfloat16
    fp8 = mybir.dt.float8e4
    nc = bacc.Bacc("TRN2", target_bir_lowering=False, debug=False, num_devices=NCORES)
    weTb_ap = nc.dram_tensor("weTb", [nch, 128, WD], bf16, kind="ExternalInput").ap()
    ATb_ap = nc.dram_tensor("ATb", [8, 128, WD], bf16, kind="ExternalInput").ap()
    HBp_ap = nc.dram_tensor("HBp", [NE, WD], bf16, kind="ExternalInput").ap()
    oh_ap = nc.dram_tensor("oh", [KC, B * 128], fp8, kind="ExternalInput").ap()
    out_ap = nc.dram_tensor("out", [B * 128, WD], f16, kind="ExternalOutput").ap()

    with tile.TileContext(nc) as tc:
        with tc.tile_pool(name="at", bufs=1) as atp, \
             tc.tile_pool(name="wet", bufs=6) as wetp, \
             tc.tile_pool(name="pt", bufs=nch) as ptp, \
             tc.tile_pool(name="sl", bufs=1) as slp, \
             tc.tile_pool(name="ohp", bufs=4) as ohp, \
             tc.tile_pool(name="ob", bufs=4) as obp, \
             tc.tile_pool(name="obd", bufs=8) as obdp, \
             tc.tile_pool(name="ps_pre", bufs=4, space="PSUM") as pspre, \
             tc.tile_pool(name="ps_exp", bufs=4, space="PSUM") as psexp:
            # fine-grained first loads: the k0 slices PG(0) touches first,
            # spread across the three DMA queues
            wb0 = wetp.tile([128, WD], bf16, tag="wet", name="wb")
            nc.sync.dma_start(wb0[:, 0:128], weTb_ap[0, :, 0:128])
            at = atp.tile([128, 8 * WD], bf16)
            nc.scalar.dma_start(at[:, 0:512], ATb_ap[0, :, 0:512])
            nc.gpsimd.dma_start(at[:, 512:1024], ATb_ap[0, :, 512:1024])
            nc.sync.dma_start(wb0[:, 128:1024], weTb_ap[0, :, 128:1024])
            at_engs = [nc.scalar, nc.gpsimd, nc.sync]
            for k in range(1, 8):
                at_engs[k % 3].dma_start(at[:, k * WD:(k + 1) * WD], ATb_ap[k])
            wbs = {0: wb0}
            if nch > 1:
                wbs[1] = wetp.tile([128, WD], bf16, tag="wet", name="wb")
                nc.sync.dma_start(wbs[1][:], weTb_ap[1])
            # persistent expansion-rhs tiles; HB stamped once in rows 0..39
            slots = []
            for s in range(NSLOT):
                sl = slp.tile([128, WD], bf16, tag=f"slot{s}", name=f"slot{s}")
                nc.gpsimd.dma_start(sl[0:NE, :], HBp_ap[:])
                slots.append(sl)

            def p_gemm(wb):
                # k-major: the startup chunk consumes AT slices in DMA order
                pp = [pspre.tile([128, 512], f32, space="PSUM", tag="pp", name="pp")
                      for _ in range(2)]
                for k in range(8):
                    for n in range(2):
                        nc.tensor.matmul(
                            pp[n][:], wb[:, k * 128:(k + 1) * 128],
                            at[:, k * WD + n * 512: k * WD + (n + 1) * 512],
                            start=(k == 0), stop=(k == 7))
                return pp

            def p_copy(pp, P):
                nc.vector.tensor_copy(P[:, 0:512], pp[0][:])
                nc.scalar.copy(P[:, 512:1024], pp[1][:])

            ptiles = []
            oh_tiles = {}

            def load_oh(b):
                if 0 <= b < B and b % GB == 0 and (b // GB) not in oh_tiles:
                    t = ohp.tile([KC, GB * 128], fp8, tag="oh", name="oht")
                    hi = min((b + GB) * 128, B * 128)
                    nc.sync.dma_start(t[:, 0:hi - b * 128], oh_ap[:, b * 128:hi])
                    oh_tiles[b // GB] = t

            def regroup_pair(p):
                # stage bin pair p's 64 P rows into rows 40..103 of its slot
                sl = slots[p % NSLOT]
                r0 = 64 * (p % 2)
                nc.gpsimd.dma_start(sl[NE:KC, :], ptiles[p // 2][r0:r0 + 64, :])

            RELU = mybir.ActivationFunctionType.Relu

            def relu_eng(eng, dst, pe):
                if eng is nc.scalar:
                    nc.scalar.activation(dst, pe[:], RELU)
                else:
                    eng.tensor_scalar_max(dst, pe[:], 0.0)

            def expand(b, ob, obcol, drain):
                load_oh(b + 2 * GB if b % GB == 0 else -1)
                oht = oh_tiles[b // GB]
                col0 = (b % GB) * 128
                sl = slots[(b // 2) % NSLOT]
                e0, e1 = (nc.scalar, nc.vector) if b % 2 == 0 else (nc.vector, nc.scalar)
                for n in range(2):
                    # drain phase: P-GEMM PSUM banks are free -- borrow them
                    pool = psexp if (not drain or n == 0) else pspre
                    pe = pool.tile([128, 512], f32, space="PSUM",
                                   tag="pe" if pool is psexp else "pp",
                                   name="pe")
                    nc.tensor.matmul(pe[:], oht[:, col0:col0 + 128],
                                     sl[0:KC, n * 512:(n + 1) * 512],
                                     start=True, stop=True)
                    relu_eng(e0 if n == 0 else e1,
                             ob[:, obcol + n * 512: obcol + (n + 1) * 512], pe)

            # 2-deep software pipeline: prologue runs P-GEMM for chunks 0/1
            # and pre-stages pairs 0/1; iteration j issues P-GEMM(j+2),
            # regroups for chunk j+1's pairs, then expansion(j).
            load_oh(0)
            load_oh(GB)
            for j in range(min(2, nch)):
                pp = p_gemm(wbs[j])
                P = ptp.tile([128, WD], bf16, tag="pt", name="P")
                p_copy(pp, P)
                ptiles.append(P)
                if j == 0:
                    regroup_pair(0)
                    regroup_pair(1)

            for j in range(nch):
                if j + 2 < nch:
                    wbn = wetp.tile([128, WD], bf16, tag="wet", name="wb")
                    nc.sync.dma_start(wbn[:], weTb_ap[j + 2])
                    pp = p_gemm(wbn)
                    P = ptp.tile([128, WD], bf16, tag="pt", name="P")
                    p_copy(pp, P)
                    ptiles.append(P)
                if j + 1 < nch:
                    regroup_pair(2 * (j + 1))
                    regroup_pair(2 * (j + 1) + 1)
                drain = j + 2 >= nch
                b0 = 4 * j
                nb = min(4, B - b0)
                if not drain:
                    # pair-merged stores: one DMA per 2 bins (512KB), off
                    # the relu engines (sync/gpsimd alternate); pair
                    # granularity keeps the end-of-stream store backlog small
                    for h in range(0, nb, 2):
                        np_ = min(2, nb - h)
                        ob = obp.tile([128, 2 * WD], f16, tag="ob", name="ob")
                        for q in range(np_):
                            expand(b0 + h + q, ob, q * WD, False)
                        dram = out_ap[(b0 + h) * 128:(b0 + h + np_) * 128, :]\
                            .rearrange("(i p) f -> p i f", p=128)
                        eng = nc.sync if ((b0 + h) // 2) % 2 == 0 else nc.gpsimd
                        eng.dma_start(dram, ob[:, 0:np_ * WD].rearrange(
                            "p (i f) -> p i f", f=WD))
                else:
                    # drain: per-bin tiles + stores for a granular tail
                    for q in range(nb):
                        ob = obdp.tile([128, WD], f16, tag="obd", name="obd")
                        expand(b0 + q, ob, 0, True)
                        eng = nc.sync if q % 2 == 0 else nc.gpsimd
                        eng.dma_start(out_ap[(b0 + q) * 128:(b0 + q + 1) * 128, :],
                                      ob[:])
    nc.compile()
    return nc


def kernel(word_emb, char_ids, word_ids, E, W_ih, b_ih, b_hh, W_lin, b_lin,
           _timing=None, _trace_cores=None, _sim_core=None):
    word_emb = np.asarray(word_emb, np.float32)
    char_ids = np.asarray(char_ids, np.int32)
    word_ids = np.asarray(word_ids, np.int32)
    E = np.asarray(E, np.float32)
    W_ih = np.asarray(W_ih, np.float32)
    b_ih = np.asarray(b_ih, np.float32)
    b_hh = np.asarray(b_hh, np.float32)
    W_lin = np.asarray(W_lin, np.float32)
    b_lin = np.asarray(b_lin, np.float32)

    T = char_ids.shape[0]
    NW = word_emb.shape[0]

    HBp = _hb_table(E, W_ih, b_ih, b_hh, W_lin, b_lin)
    A = np.ascontiguousarray(W_lin[:, :WD])

    bin_of_word, slot_of_word, nbins = _pack_bins(word_ids, NW)
    B = nbins // NCORES
    nch = (B * WS) // 128
    assert B * WS % 128 == 0

    # char ordering: grouped by bin, word-major (slot) inside
    cb = bin_of_word[word_ids]
    cs = slot_of_word[word_ids]
    ckey = cb.astype(np.int64) * 64 + cs
    corder = np.argsort(ckey, kind="stable")
    per_bin = np.bincount(cb, minlength=nbins)
    assert per_bin.max() <= CAP
    bstart = np.concatenate([[0], np.cumsum(per_bin)])
    pos = np.arange(T) - bstart[cb[corder]]        # char position within bin
    ATb = np.ascontiguousarray(A.T.reshape(8, 128, WD)).astype(NPBF)
    HBq = HBp.astype(NPBF)

    cb_s = cb[corder]
    cid_s = char_ids[corder]
    cslot_s = cs[corder]

    in_maps = []
    origs = []
    for m in range(NCORES):
        b_lo, b_hi = m * B, (m + 1) * B
        # word list in (bin, slot) order with dummy padding to WS per bin
        wlist = np.full(B * WS, -1, np.int64)
        sel_w = np.nonzero((bin_of_word >= b_lo) & (bin_of_word < b_hi))[0]
        wslot = (bin_of_word[sel_w].astype(np.int64) - b_lo) * WS + slot_of_word[sel_w]
        wlist[wslot] = sel_w
        rows = np.where((wlist >= 0)[:, None],
                        word_emb[np.maximum(wlist, 0)], 0.0)  # [B*WS, WD]
        # weTb[c, p, k*128+w] = rows[c*128+w, k*128+p]
        weTb = np.ascontiguousarray(
            rows.reshape(nch, 128, 8, 128).transpose(0, 3, 2, 1)
        ).astype(NPBF).reshape(nch, 128, WD)

        msk = (cb_s >= b_lo) & (cb_s < b_hi)
        lb = cb_s[msk].astype(np.int64) - b_lo
        col = lb * 128 + pos[msk]
        oh = np.zeros((KC, B * 128), NP8)
        oh[cid_s[msk], col] = 1.0
        oh[NE + (lb % 2) * WS + cslot_s[msk], col] = 1.0
        orig = np.full(B * 128, -1, np.int64)
        orig[col] = corder[msk]
        in_maps.append({"weTb": weTb, "ATb": ATb, "HBp": HBq, "oh": oh})
        origs.append(orig)

    nc = _build_program(B, nch)

    if _sim_core is not None:
        from concourse.bass_interp import CoreSim
        sim = CoreSim(nc, trace=False)
        for k, v in in_maps[_sim_core].items():
            sim.tensor(k)[:] = v
        sim.simulate(check_with_hw=False)
        o = np.asarray(sim.tensor("out"), np.float32)
        out = np.full((T, WD), np.nan, np.float32)
        v = origs[_sim_core] >= 0
        out[origs[_sim_core][v]] = o[v]
        return out

    kwargs = {}
    if _trace_cores is not None:
        kwargs = dict(trace=True, trace_cores=_trace_cores)
    res = run_bass_kernel_spmd(nc, in_maps, core_ids=list(range(NCORES)), **kwargs)
    if _timing is not None:
        _timing["exec_time_ns"] = res.exec_time_ns
        _timing["results"] = res

    out = np.empty((T, WD), np.float32)
    for m in range(NCORES):
        o = np.asarray(res.results[m]["out"], np.float32)
        v = origs[m] >= 0
        out[origs[m][v]] = o[v]
    return out
